# revision 9
# baseline (speedup 1.0000x reference)
"""Trainium2 Bass kernel v3: 2-layer GRU (T=512, B=128, IN=64, H=1024) +
time-distributed linear (OUT=64) on 8 NeuronCores.

v3 over v2 (15.8ms):
- Recurrent matmuls use 4-way PE COLUMN TILING (tile_position=(0,32j)):
  each step's h@W_hh has only M=16 output rows, so a single matmul uses
  16/128 PE columns. Four matmuls in distinct 32-column groups stream
  CONCURRENTLY (measured ~4ns stagger on HW), quadrupling effective PE
  throughput for the recurrence vs serialized fp8-DoubleRow (col tiling
  and DoubleRow are mutually exclusive; 4x concurrency > 2x DR).
- All weights bf16 (fp8 recurrence was numerically marginal at the 2e-2
  gate and run-variable; bf16 gives ~4x more margin AND is faster here).
- h_new -> hT transposes moved from PE (transpose-mode, which forces a
  tiling-mode switch + array drain every step) to DVE StreamTranspose
  (32x32 blocks, SBUF->SBUF). The PE stays in one (128,32) tiling mode
  for the whole scan: no drains, shorter idle gaps, HAM stays warm.
- gi/bias folds stay as identity matmuls but with K padded to 128 so
  they share the (128,32) tile mode (16-row identities lower to (32,32)
  mode and would force a mode switch per chunk).

Sharding: data-parallel over batch (16 per core), weights replicated.
"""

import sys

for _p in ("/opt/trn_rl_repo", "/root/.axon_site/_ro/trn_rl_repo"):
    if _p not in sys.path:
        sys.path.insert(0, _p)

import numpy as np
import ml_dtypes

import concourse.bass as bass
import concourse.mybir as mybir
import concourse.tile as tile
from concourse.bass import ds
from concourse.bass_utils import run_bass_kernel_spmd

F32 = mybir.dt.float32
BF16 = mybir.dt.bfloat16
AF = mybir.ActivationFunctionType

N_CORES = 8
B, IN, H, OUT = 128, 64, 1024, 64
BL = B // N_CORES          # 16
G3 = 3 * H                 # 3072
HALF = H // 2              # 512
NCH = G3 // 512            # 6
KT = H // 128              # 8
import os
CG = ([0] * 6 if os.environ.get('SCAN_TP') == '0'
      else [0, 1, 2, 3, 0, 1])    # chunk -> PE column group


# ---- walrus workaround: split the TileContext closing drain's waits ----
def _patched_drain_and_barrier(self, tick_clock, wait_clock):
    from concourse.vector_clock import ScopedClock
    drain_inst = self.nc.sync.drain()
    wait_clock.add_sem_waits(
        drain_inst.ins, ScopedClock({None: tick_clock.global_clock}))
    mi = drain_inst.ins
    si = mi.sync_info
    waits = list(si.on_wait) if (si is not None and si.on_wait) else []
    if len(waits) > 1:
        si.on_wait = waits[:1]
        mi.sync_info = si
        for w in waits[1:]:
            extra = self.nc.sync.drain()
            emi = extra.ins
            esi = emi.sync_info
            if esi is None:
                esi = mybir.SyncInfo(on_wait=[], on_update=[])
            esi.on_wait = [w]
            emi.sync_info = esi
    self.nc.all_engine_barrier()
    assert self.sems is not None
    popped = self.nc._tile_sem_poison_stack.pop()
    assert popped is self._sem_poison
    self.nc.clear_and_free_semaphores(list(self.sems.allocated().values()))
    self.nc.all_engine_barrier()


tile.TileContext._drain_and_barrier = _patched_drain_and_barrier


def _perm_rows():
    r = np.arange(0, H)
    z = np.arange(H, 2 * H)
    n = np.arange(2 * H, 3 * H)
    return np.concatenate([
        r[:HALF], z[:HALF], n[:HALF], r[HALF:], z[HALF:], n[HALF:]])


PERM = _perm_rows()


def _build_program(T):
    NTOK = T * BL
    nc = bass.Bass("TRN2", target_bir_lowering=False, debug=False)

    xT = nc.declare_dram_parameter("xT", [IN, NTOK + 128], BF16, isOutput=False)
    w_ih0T = nc.declare_dram_parameter("w_ih0T", [IN, G3], BF16, isOutput=False)
    w_hh0T = nc.declare_dram_parameter("w_hh0T", [KT, 128, G3], BF16, isOutput=False)
    w_ih1T = nc.declare_dram_parameter("w_ih1T", [KT, 128, G3], BF16, isOutput=False)
    w_hh1T = nc.declare_dram_parameter("w_hh1T", [KT, 128, G3], BF16, isOutput=False)
    w_linT = nc.declare_dram_parameter("w_linT", [KT, 128, OUT], BF16, isOutput=False)
    gbias0 = nc.declare_dram_parameter("gbias0", [128, G3], F32, isOutput=False)
    gbias1 = nc.declare_dram_parameter("gbias1", [128, G3], F32, isOutput=False)
    bhn0d = nc.declare_dram_parameter("bhn0", [128, H], BF16, isOutput=False)
    bhn1d = nc.declare_dram_parameter("bhn1", [128, H], BF16, isOutput=False)
    identd = nc.declare_dram_parameter("identB", [128, BL], BF16, isOutput=False)
    blind = nc.declare_dram_parameter("blin", [128, OUT], F32, isOutput=False)
    y = nc.declare_dram_parameter("y", [NTOK, OUT], F32, isOutput=True)

    with tile.TileContext(nc) as tc:
        with tc.tile_pool(name="dram", bufs=1, space="DRAM") as dpool:
            gi0 = dpool.tile([T + 2, BL, G3], BF16, tag="gi0", name="gi0")
            gi1 = dpool.tile([T + 2, BL, G3], BF16, tag="gi1", name="gi1")
            h1T = dpool.tile([T, KT, 128, BL], BF16, tag="h1T", name="h1T")
            h2T = dpool.tile([T, KT, 128, BL], BF16, tag="h2T", name="h2T")

            # ---------------- phase 1: gi0 ----------------
            with tc.tile_pool(name="p1", bufs=1) as cp, \
                 tc.tile_pool(name="p1w", bufs=3) as wp, \
                 tc.tile_pool(name="p1ps", bufs=3, space="PSUM") as pp:
                wih0 = cp.tile([IN, G3], BF16, tag="wih0", name="wih0")
                nc.sync.dma_start(out=wih0[:, :], in_=w_ih0T[:, :])
                gb = cp.tile([128, G3], F32, tag="gb0", name="gb0")
                nc.sync.dma_start(out=gb[:, :], in_=gbias0[:, :])
                for it in range(NTOK // 128):
                    xt = wp.tile([IN, 128], BF16, tag="xt", name="xt")
                    nc.sync.dma_start(
                        out=xt[:, :], in_=xT[:, it * 128:(it + 1) * 128])
                    gsb = wp.tile([128, G3], BF16, tag="gsb", name="gsb")
                    for c in range(NCH):
                        gps = pp.tile([128, 512], F32, tag="gps", name="gps")
                        nc.tensor.matmul(
                            gps[:, :], xt[:, :], wih0[:, c * 512:(c + 1) * 512],
                            start=True, stop=True)
                        nc.vector.tensor_add(
                            gsb[:, c * 512:(c + 1) * 512], gps[:, :],
                            gb[:, c * 512:(c + 1) * 512])
                    nc.sync.dma_start(
                        out=gi0[:, :, :].rearrange("t b n -> (t b) n")[
                            it * 128:(it + 1) * 128, :],
                        in_=gsb[:, :])

            # ---------------- phase 2: scan layer 0 ----------------
            _scan_layer(nc, tc, T, w_hh0T, bhn0d, identd, gi0, h1T)

            # ---------------- phase 3: gi1 ----------------
            with tc.tile_pool(name="p3", bufs=1) as cp, \
                 tc.tile_pool(name="p3w", bufs=3) as wp, \
                 tc.tile_pool(name="p3g", bufs=2) as gp, \
                 tc.tile_pool(name="p3ps", bufs=3, space="PSUM") as pp:
                wih1 = cp.tile([128, KT, G3], BF16, tag="wih1", name="wih1")
                nc.sync.dma_start(
                    out=wih1[:, :, :],
                    in_=w_ih1T[:, :, :].rearrange("k p n -> p k n"))
                gb = cp.tile([128, G3], F32, tag="gb1", name="gb1")
                nc.sync.dma_start(out=gb[:, :], in_=gbias1[:, :])
                TT = 128 // BL  # timesteps per token tile = 8
                for it in range(NTOK // 128):
                    hts = wp.tile([128, KT, 128], BF16, tag="hts", name="hts")
                    for k in range(KT):
                        nc.sync.dma_start(
                            out=hts[:, k, :].rearrange("p (t b) -> p t b", t=TT),
                            in_=h1T[it * TT:(it + 1) * TT, k, :, :].rearrange(
                                "t p b -> p t b"))
                    gsb = gp.tile([128, G3], BF16, tag="gsb1", name="gsb1")
                    for c in range(NCH):
                        gps = pp.tile([128, 512], F32, tag="gps1", name="gps1")
                        for k in range(KT):
                            nc.tensor.matmul(
                                gps[:, :], hts[:, k, :],
                                wih1[:, k, c * 512:(c + 1) * 512],
                                start=(k == 0), stop=(k == KT - 1))
                        nc.vector.tensor_add(
                            gsb[:, c * 512:(c + 1) * 512], gps[:, :],
                            gb[:, c * 512:(c + 1) * 512])
                    nc.sync.dma_start(
                        out=gi1[:, :, :].rearrange("t b n -> (t b) n")[
                            it * 128:(it + 1) * 128, :],
                        in_=gsb[:, :])

            # ---------------- phase 4: scan layer 1 ----------------
            _scan_layer(nc, tc, T, w_hh1T, bhn1d, identd, gi1, h2T)

            # ---------------- phase 5: linear ----------------
            with tc.tile_pool(name="p5", bufs=1) as cp, \
                 tc.tile_pool(name="p5w", bufs=3) as wp, \
                 tc.tile_pool(name="p5ps", bufs=3, space="PSUM") as pp:
                wlin = cp.tile([128, KT, OUT], BF16, tag="wlin", name="wlin")
                nc.sync.dma_start(
                    out=wlin[:, :, :],
                    in_=w_linT[:, :, :].rearrange("k p n -> p k n"))
                bl = cp.tile([128, OUT], F32, tag="bl", name="bl")
                nc.sync.dma_start(out=bl[:, :], in_=blind[:, :])
                TT = 128 // BL
                for it in range(NTOK // 128):
                    hts = wp.tile([128, KT, 128], BF16, tag="hts5", name="hts5")
                    for k in range(KT):
                        nc.sync.dma_start(
                            out=hts[:, k, :].rearrange("p (t b) -> p t b", t=TT),
                            in_=h2T[it * TT:(it + 1) * TT, k, :, :].rearrange(
                                "t p b -> p t b"))
                    ops = pp.tile([128, OUT], F32, tag="lps", name="lps")
                    for k in range(KT):
                        nc.tensor.matmul(
                            ops[:, :], hts[:, k, :], wlin[:, k, :],
                            start=(k == 0), stop=(k == KT - 1))
                    osb = wp.tile([128, OUT], F32, tag="osb", name="osb")
                    nc.vector.tensor_add(osb[:, :], ops[:, :], bl[:, :])
                    nc.sync.dma_start(
                        out=y[it * 128:(it + 1) * 128, :], in_=osb[:, :])
    _split_excess_waits(nc)
    return nc


_WAIT_LIMIT = 1


def _split_excess_waits(nc):
    """walrus CoreV3 allows only ~2 sync waits per instruction; hoist the
    excess onto NoOp instructions inserted just before, on the same engine."""
    for bb in nc.main_func.blocks:
        insts = list(bb.instructions)
        out, n_extra = [], 0
        for inst in insts:
            si = inst.sync_info
            waits = list(si.on_wait) if (si is not None and si.on_wait) else []
            if len(waits) > _WAIT_LIMIT:
                keep = waits[-_WAIT_LIMIT:]
                excess = waits[:-_WAIT_LIMIT]
                for j in range(0, len(excess), _WAIT_LIMIT):
                    nop = mybir.InstNoOp(
                        name=f"{inst.name}-w{j}-{n_extra}", ins=[], outs=[])
                    nop.engine = inst.engine
                    nop.sync_info = mybir.SyncInfo(
                        on_wait=excess[j:j + _WAIT_LIMIT], on_update=[])
                    out.append(nop)
                    n_extra += 1
                si.on_wait = keep
                inst.sync_info = si
            out.append(inst)
        if n_extra:
            bb.instructions = out
    mx = max((len(i.sync_info.on_wait) if i.sync_info and i.sync_info.on_wait else 0)
             for bb in nc.main_func.blocks for i in bb.instructions)
    print("[split_waits] max on_wait after pass:", mx)


def _scan_layer(nc, tc, T, w_hhT_dram, bhn_dram, ident_dram, gi_dram, hT_dram):
    with tc.tile_pool(name="scst", bufs=1) as st, \
         tc.tile_pool(name="scps", bufs=1, space="PSUM") as pp:
        whh = st.tile([128, KT, G3], BF16, tag="whh", name="whh")
        nc.sync.dma_start(
            out=whh[:, :, :],
            in_=w_hhT_dram[:, :, :].rearrange("k p n -> p k n"))
        bhn = st.tile([128, H], BF16, tag="bhn", name="bhn")
        nc.sync.dma_start(out=bhn[:, :], in_=bhn_dram[:, :])
        ident = st.tile([128, BL], BF16, tag="ident", name="ident")
        nc.sync.dma_start(out=ident[:, :], in_=ident_dram[:, :])

        # hT: h state transposed (stationary operand), padded to 32 batch cols
        hT = [st.tile([128, KT, 32], BF16, tag=f"hT{i}", name=f"hT{i}")
              for i in range(2)]
        # h state batch-major, padded to 32 partitions. Layout
        # [part, half(2), quad(4), kin(4), 32]: element (h,q,k,e) holds
        # H-dim 512h + 128k + 32q + e, so each DVE block-transpose input
        # (fixed half+quad) is a contiguous 128-col run of 32x32 blocks.
        hF = [st.tile([32, 2, 4, 4, 32], BF16, tag=f"hF{i}", name=f"hF{i}")
              for i in range(2)]
        # gi slots padded to 128 partitions (rows 16.. stay zero) so the
        # identity fold-in matmuls stay in (128,32) tile mode
        giA = [st.tile([128, G3], BF16, tag=f"gi{i}", name=f"gi{i}")
               for i in range(3)]
        gps = [pp.tile([128, 512], F32, tag=f"gps{c}", name=f"gps{c}")
               for c in range(NCH)]
        rga = [st.tile([BL, HALF], BF16, tag=f"rg{i}", name=f"rg{i}") for i in range(2)]
        zga = [st.tile([BL, HALF], BF16, tag=f"zg{i}", name=f"zg{i}") for i in range(2)]
        nga = [st.tile([BL, HALF], BF16, tag=f"ng{i}", name=f"ng{i}") for i in range(2)]
        tmp = [st.tile([BL, HALF], BF16, tag=f"tmp{i}", name=f"tmp{i}") for i in range(2)]

        nc.vector.memset(hT[0][:, :, :], 0.0)
        for i in range(2):
            nc.vector.memset(hF[i][:, :, :, :, :], 0.0)
        for s in range(3):
            nc.vector.memset(giA[s][:, :], 0.0)
        for s in range(2):
            nc.sync.dma_start(
                out=giA[s][0:BL, :],
                in_=gi_dram[s:s + 1, :, :].rearrange("o b n -> (o b) n"))

        def step(t_reg, toff, par):
            hin, hout = hT[par], hT[1 - par]
            hfin, hfout = hF[par], hF[1 - par]
            gi = giA[toff % 3]

            # identity row: fold gi (r/z chunks) or b_hh-n (n chunks) into
            # each chunk's psum slice; K padded to 128 to stay in (128,32)
            # tile mode. Column groups rotate 0,1,2,3,0,1 so up to 4 chunks
            # stream concurrently in distinct PE column groups.
            for c in range(NCH):
                half, j = c // 3, c % 3
                g = 32 * CG[c]
                if j < 2:
                    rhs = gi[:, c * 512:(c + 1) * 512]
                else:
                    rhs = bhn[:, half * HALF:(half + 1) * HALF]
                nc.tensor.matmul(gps[c][g:g + BL, :], ident[:, :], rhs,
                                 start=True, stop=False,
                                 tile_position=(0, g))
            for k in range(KT):
                for c in range(NCH):
                    g = 32 * CG[c]
                    nc.tensor.matmul(
                        gps[c][g:g + BL, :],
                        hin[:, k, 0:BL],
                        whh[:, k, c * 512:(c + 1) * 512],
                        start=False, stop=(k == KT - 1),
                        tile_position=(0, g))

            if os.environ.get('SCAN_NOCHAIN') == '1':
                _chain_halves = []
            else:
                _chain_halves = range(2)
            for half in _chain_halves:
                cr, cz, cn = 3 * half, 3 * half + 1, 3 * half + 2
                go = half * 3 * 512
                pr = gps[cr][32 * CG[cr]:32 * CG[cr] + BL, :]
                pz = gps[cz][32 * CG[cz]:32 * CG[cz] + BL, :]
                pn = gps[cn][32 * CG[cn]:32 * CG[cn] + BL, :]
                # stride-permuted view: free iteration order (kin, quad, e)
                # matches the psum gate columns' contiguous H order
                hfi = hfin[0:BL, half:half + 1, :, :, :].rearrange(
                    "p o q k e -> p o k q e")
                hfo = hfout[0:BL, half:half + 1, :, :, :].rearrange(
                    "p o q k e -> p o k q e")
                nc.scalar.activation(rga[half][:, :], pr, AF.Sigmoid)
                nc.scalar.activation(zga[half][:, :], pz, AF.Sigmoid)
                nc.vector.tensor_mul(tmp[half][:, :], rga[half][:, :], pn)
                nc.vector.tensor_add(tmp[half][:, :], tmp[half][:, :],
                                     gi[0:BL, go + 1024:go + 1536])
                nc.scalar.activation(nga[half][:, :], tmp[half][:, :], AF.Tanh)
                nc.vector.tensor_sub(tmp[half][:, :], hfi, nga[half][:, :])
                nc.vector.tensor_mul(tmp[half][:, :], zga[half][:, :],
                                     tmp[half][:, :])
                nc.vector.tensor_add(hfo, nga[half][:, :], tmp[half][:, :])
                # hT refresh: 4 DVE 32x32 block-transpose ops, one per
                # destination partition quadrant (SBUF->SBUF, PE untouched)
                for r in ([] if os.environ.get('SCAN_NOTR') == '1' else range(4)):
                    nc.vector.transpose(
                        hout[32 * r:32 * r + 32, 4 * half:4 * half + 4, :],
                        hfout[0:32, half, r, :, :])

            nc.sync.dma_start(
                out=hT_dram[ds(t_reg + toff, 1), :, :, :].rearrange(
                    "o k p b -> p (o k) b"),
                in_=hout[:, :, 0:BL])
            nc.sync.dma_start(
                out=giA[toff % 3][0:BL, :],
                in_=gi_dram[ds(t_reg + toff + 2, 1), :, :].rearrange(
                    "o b n -> (o b) n"))

        with tc.For_i(0, T, 2) as t:
            step(t, 0, 0)
            step(t, 1, 1)


# ---------------- host-side wrapper ----------------
def _prep_inputs(x, w_ih0, w_hh0, b_ih0, b_hh0, w_ih1, w_hh1, b_ih1, b_hh1,
                 w_lin, b_lin, T):
    NTOK = T * BL
    bf = ml_dtypes.bfloat16

    def prep_layer(w_ih, w_hh, b_ih, b_hh):
        w_ihP = np.asarray(w_ih)[PERM, :]          # [3H, in]
        w_hhP = np.asarray(w_hh)[PERM, :]          # [3H, H]
        b_ihP = np.asarray(b_ih)[PERM]
        b_hhP = np.asarray(b_hh)[PERM]
        # gi bias: b_ih everywhere + b_hh on r/z blocks (n gets b_hh inside r*())
        gb = b_ihP.copy()
        for blk in range(2):
            o = blk * 3 * 512
            gb[o:o + 1024] += b_hhP[o:o + 1024]    # r and z blocks
        # b_hh n-gate halves -> [H] = [n0 n1]
        bhn = np.concatenate([b_hhP[1024:1536], b_hhP[1024 + 1536:1536 + 1536]])
        w_ihT = np.ascontiguousarray(w_ihP.T).astype(bf)      # [in, 3H]
        w_hhT = np.ascontiguousarray(w_hhP.T).astype(bf)      # [H, 3H]
        w_hhT = w_hhT.reshape(KT, 128, G3)
        gbB = np.broadcast_to(gb.astype(np.float32), (128, G3)).copy()
        bhnB = np.broadcast_to(bhn.astype(bf), (128, H)).copy()
        return w_ihT, w_hhT, gbB, bhnB

    w_ih0T, w_hh0T, gb0, bhn0 = prep_layer(w_ih0, w_hh0, b_ih0, b_hh0)
    w_ih1T, w_hh1T, gb1, bhn1 = prep_layer(w_ih1, w_hh1, b_ih1, b_hh1)
    w_ih1T = w_ih1T.reshape(KT, 128, G3)
    w_linT = np.ascontiguousarray(np.asarray(w_lin).T).astype(bf).reshape(
        KT, 128, OUT)
    blinB = np.broadcast_to(np.asarray(b_lin).astype(np.float32),
                            (128, OUT)).copy()
    identB = np.eye(128, BL, dtype=bf)

    common = dict(w_ih0T=w_ih0T, w_hh0T=w_hh0T, w_ih1T=w_ih1T, w_hh1T=w_hh1T,
                  w_linT=w_linT, gbias0=gb0, gbias1=gb1, bhn0=bhn0, bhn1=bhn1,
                  blin=blinB, identB=identB)

    in_maps = []
    x = np.asarray(x)
    for c in range(N_CORES):
        xs = x[:T, c * BL:(c + 1) * BL, :]          # [T, BL, IN]
        xT = np.zeros((IN, NTOK + 128), dtype=bf)
        xT[:, :NTOK] = xs.reshape(NTOK, IN).T.astype(bf)
        m = dict(common)
        m["xT"] = xT
        in_maps.append(m)
    return in_maps


_NC_CACHE = {}


def get_program(T):
    if T not in _NC_CACHE:
        _NC_CACHE[T] = _build_program(T)
    return _NC_CACHE[T]


def run(x, w_ih0, w_hh0, b_ih0, b_hh0, w_ih1, w_hh1, b_ih1, b_hh1,
        w_lin, b_lin, T=512, trace=False):
    in_maps = _prep_inputs(x, w_ih0, w_hh0, b_ih0, b_hh0, w_ih1, w_hh1,
                           b_ih1, b_hh1, w_lin, b_lin, T)
    nc = get_program(T)
    res = run_bass_kernel_spmd(nc, in_maps, core_ids=list(range(N_CORES)),
                               trace=trace)
    NTOK = T * BL
    out = np.empty((T, B, OUT), dtype=np.float32)
    for c in range(N_CORES):
        out[:, c * BL:(c + 1) * BL, :] = res.results[c]["y"].reshape(
            T, BL, OUT)
    return out, res


def kernel(**inputs):
    out, _ = run(**inputs)
    return out


# revision 18
# speedup vs baseline: 1.9015x; 1.9015x over previous
"""Trainium2 Bass kernel v4: 2-layer GRU (T=512, B=128, IN=64, H=1024) +
time-distributed linear (OUT=64) on 8 NeuronCores.

v4 over v3 (18.1ms) / v2 (15.8ms):
- FUSED two-layer scan: one loop runs layer-0 step s and layer-1 step
  s-2 per slot, so each layer's elementwise chain + hT refresh hides
  under the other layer's matmul streams (the ~10us/step serial-chain
  stall dominated v2/v3, with HAM re-throttling the PE to half clock in
  every gap).
- Layer-1's input gates (h1 @ W_ih1) accumulate per-step into the same
  PSUM groups as its recurrence, reading the h1T tile layer-0 produced
  two slots earlier straight from SBUF. The batched gi1 phase and all
  h1T DRAM traffic are gone; those matmuls depend only on lag-2 state,
  so they lead each slot and paper over the previous slot's chain tail.
- All gate matmuls use 4-way PE column tiling (tile_position=(0,32j)).
  Gate chunks [r0 r1 z0 z1 | n0 n1 | gin0 gin1] map to 3 PSUM banks x 4
  column groups per layer; consecutive matmuls rotate groups so up to 4
  streams run concurrently on the array.
- Wide elementwise: ONE sigmoid over a whole PSUM bank ([128,512] = all
  four r/z chunks, 112 lanes instead of 16) and [48,512] n-path ops
  covering both halves (stacked partition groups).
- h_new -> hT transposes on the DMA X-bar (dma_start_transpose, 16x128
  bf16 tiles): zero DVE/PE cost, runs on otherwise-idle DMA engines.
- All weights bf16 (v2's fp8 recurrence was numerically marginal at the
  2e-2 gate and run-variable; bf16 measures ~5e-3).

Sharding: data-parallel over batch (16 per core), weights replicated.
"""

import sys

for _p in ("/opt/trn_rl_repo", "/root/.axon_site/_ro/trn_rl_repo"):
    if _p not in sys.path:
        sys.path.insert(0, _p)

import numpy as np
import ml_dtypes

import concourse.bass as bass
import concourse.mybir as mybir
import concourse.tile as tile
from concourse.bass import ds
from concourse.bass_utils import run_bass_kernel_spmd

F32 = mybir.dt.float32
BF16 = mybir.dt.bfloat16
AF = mybir.ActivationFunctionType

N_CORES = 8
B, IN, H, OUT = 128, 64, 1024, 64
BL = B // N_CORES          # 16
G3 = 3 * H                 # 3072
HALF = H // 2              # 512
KT = H // 128              # 8


# ---- walrus workaround: split the TileContext closing drain's waits ----
def _patched_drain_and_barrier(self, tick_clock, wait_clock):
    from concourse.vector_clock import ScopedClock
    drain_inst = self.nc.sync.drain()
    wait_clock.add_sem_waits(
        drain_inst.ins, ScopedClock({None: tick_clock.global_clock}))
    mi = drain_inst.ins
    si = mi.sync_info
    waits = list(si.on_wait) if (si is not None and si.on_wait) else []
    if len(waits) > 1:
        si.on_wait = waits[:1]
        mi.sync_info = si
        for w in waits[1:]:
            extra = self.nc.sync.drain()
            emi = extra.ins
            esi = emi.sync_info
            if esi is None:
                esi = mybir.SyncInfo(on_wait=[], on_update=[])
            esi.on_wait = [w]
            emi.sync_info = esi
    self.nc.all_engine_barrier()
    assert self.sems is not None
    popped = self.nc._tile_sem_poison_stack.pop()
    assert popped is self._sem_poison
    self.nc.clear_and_free_semaphores(list(self.sems.allocated().values()))
    self.nc.all_engine_barrier()


tile.TileContext._drain_and_barrier = _patched_drain_and_barrier


def _build_program(T):
    NTOK = T * BL
    nc = bass.Bass("TRN2", target_bir_lowering=False, debug=False)

    xT = nc.declare_dram_parameter("xT", [IN, NTOK + 128], BF16, isOutput=False)
    w_ih0T = nc.declare_dram_parameter("w_ih0T", [IN, G3], BF16, isOutput=False)
    w_hh0T = nc.declare_dram_parameter("w_hh0T", [KT, 128, G3], BF16, isOutput=False)
    w_ih1T = nc.declare_dram_parameter("w_ih1T", [KT, 128, G3], BF16, isOutput=False)
    w_hh1T = nc.declare_dram_parameter("w_hh1T", [KT, 128, G3], BF16, isOutput=False)
    w_linT = nc.declare_dram_parameter("w_linT", [KT, 128, OUT], BF16, isOutput=False)
    gbias0 = nc.declare_dram_parameter("gbias0", [128, G3], F32, isOutput=False)
    gbrz1d = nc.declare_dram_parameter("gbrz1", [128, 2 * H], BF16, isOutput=False)
    bhn0d = nc.declare_dram_parameter("bhn0", [128, H], BF16, isOutput=False)
    bhn1d = nc.declare_dram_parameter("bhn1", [128, H], BF16, isOutput=False)
    gbn1d = nc.declare_dram_parameter("gbn1", [128, H], BF16, isOutput=False)
    identd = nc.declare_dram_parameter("identB", [128, BL], BF16, isOutput=False)
    blind = nc.declare_dram_parameter("blin", [128, OUT], F32, isOutput=False)
    y = nc.declare_dram_parameter("y", [NTOK, OUT], F32, isOutput=True)

    with tile.TileContext(nc) as tc:
        with tc.tile_pool(name="dram", bufs=1, space="DRAM") as dpool:
            gi0 = dpool.tile([T + 12, BL, G3], BF16, tag="gi0", name="gi0")
            h2F = dpool.tile([T, 48, HALF], BF16, tag="h2F", name="h2F")

            # ---------------- phase 1: gi0 = x @ w_ih0 + biases ----------
            with tc.tile_pool(name="p1", bufs=1) as cp, \
                 tc.tile_pool(name="p1w", bufs=3) as wp, \
                 tc.tile_pool(name="p1ps", bufs=3, space="PSUM") as pp:
                wih0 = cp.tile([IN, G3], BF16, tag="wih0", name="wih0")
                nc.sync.dma_start(out=wih0[:, :], in_=w_ih0T[:, :])
                gb = cp.tile([128, G3], F32, tag="gb0", name="gb0")
                nc.sync.dma_start(out=gb[:, :], in_=gbias0[:, :])
                for it in range(NTOK // 128):
                    xt = wp.tile([IN, 128], BF16, tag="xt", name="xt")
                    nc.sync.dma_start(
                        out=xt[:, :], in_=xT[:, it * 128:(it + 1) * 128])
                    gsb = wp.tile([128, G3], BF16, tag="gsb", name="gsb")
                    for c in range(6):
                        gps = pp.tile([128, 512], F32, tag="gps", name="gps")
                        nc.tensor.matmul(
                            gps[:, :], xt[:, :], wih0[:, c * 512:(c + 1) * 512],
                            start=True, stop=True)
                        nc.vector.tensor_add(
                            gsb[:, c * 512:(c + 1) * 512], gps[:, :],
                            gb[:, c * 512:(c + 1) * 512])
                    nc.sync.dma_start(
                        out=gi0[:, :, :].rearrange("t b n -> (t b) n")[
                            it * 128:(it + 1) * 128, :],
                        in_=gsb[:, :])

            # ---------------- phase 2: fused 2-layer scan ----------------
            _fused_scan(nc, tc, T, w_hh0T, w_hh1T, w_ih1T, gbrz1d, bhn0d,
                        bhn1d, gbn1d, identd, gi0, h2F)

            # ---------------- phase 3: linear ----------------
            with tc.tile_pool(name="p5", bufs=1) as cp, \
                 tc.tile_pool(name="p5w", bufs=3) as wp, \
                 tc.tile_pool(name="p5ps", bufs=3, space="PSUM") as pp:
                wlin = cp.tile([128, KT, OUT], BF16, tag="wlin", name="wlin")
                nc.sync.dma_start(
                    out=wlin[:, :, :],
                    in_=w_linT[:, :, :].rearrange("k p n -> p k n"))
                bl = cp.tile([128, OUT], F32, tag="bl", name="bl")
                nc.sync.dma_start(out=bl[:, :], in_=blind[:, :])
                TT = 128 // BL
                for it in range(NTOK // 128):
                    hts = wp.tile([128, KT, 128], BF16, tag="hts5", name="hts5")
                    for uu in range(TT):
                        u = it * TT + uu
                        for hh in range(2):
                            nc.sync.dma_start_transpose(
                                hts[:, 4 * hh:4 * hh + 4,
                                    16 * uu:16 * uu + 16],
                                h2F[u:u + 1, 32 * hh:32 * hh + BL, :].rearrange(
                                    "o p n -> (o p) n"))
                    ops = pp.tile([128, OUT], F32, tag="lps", name="lps")
                    for k in range(KT):
                        nc.tensor.matmul(
                            ops[:, :], hts[:, k, :], wlin[:, k, :],
                            start=(k == 0), stop=(k == KT - 1))
                    osb = wp.tile([128, OUT], F32, tag="osb", name="osb")
                    nc.vector.tensor_add(osb[:, :], ops[:, :], bl[:, :])
                    nc.sync.dma_start(
                        out=y[it * 128:(it + 1) * 128, :], in_=osb[:, :])
    _split_excess_waits(nc)
    return nc


_WAIT_LIMIT = 1


def _split_excess_waits(nc):
    """walrus CoreV3 allows only ~2 sync waits per instruction; hoist the
    excess onto NoOp instructions inserted just before, on the same engine."""
    for bb in nc.main_func.blocks:
        insts = list(bb.instructions)
        out, n_extra = [], 0
        for inst in insts:
            si = inst.sync_info
            waits = list(si.on_wait) if (si is not None and si.on_wait) else []
            if len(waits) > _WAIT_LIMIT:
                keep = waits[-_WAIT_LIMIT:]
                excess = waits[:-_WAIT_LIMIT]
                for j in range(0, len(excess), _WAIT_LIMIT):
                    nop = mybir.InstNoOp(
                        name=f"{inst.name}-w{j}-{n_extra}", ins=[], outs=[])
                    nop.engine = inst.engine
                    nop.sync_info = mybir.SyncInfo(
                        on_wait=excess[j:j + _WAIT_LIMIT], on_update=[])
                    out.append(nop)
                    n_extra += 1
                si.on_wait = keep
                inst.sync_info = si
            out.append(inst)
        if n_extra:
            bb.instructions = out
    mx = max((len(i.sync_info.on_wait) if i.sync_info and i.sync_info.on_wait else 0)
             for bb in nc.main_func.blocks for i in bb.instructions)
    print("[split_waits] max on_wait after pass:", mx)


def _fused_scan(nc, tc, T, whh0d, whh1d, wih1d, gbrz1d, bhn0d, bhn1d, gbn1d,
                ident_dram, gi_dram, h2F_dram):
    with tc.tile_pool(name="fst", bufs=1) as st, \
         tc.tile_pool(name="fps", bufs=1, space="PSUM") as pp:
        whh0 = st.tile([128, KT, G3], BF16, tag="whh0", name="whh0")
        nc.sync.dma_start(out=whh0[:, :, :],
                          in_=whh0d[:, :, :].rearrange("k p n -> p k n"))
        whh1 = st.tile([128, KT, G3], BF16, tag="whh1", name="whh1")
        nc.sync.dma_start(out=whh1[:, :, :],
                          in_=whh1d[:, :, :].rearrange("k p n -> p k n"))
        wih1 = st.tile([128, KT, G3], BF16, tag="wih1", name="wih1")
        nc.sync.dma_start(out=wih1[:, :, :],
                          in_=wih1d[:, :, :].rearrange("k p n -> p k n"))
        gbrz1 = st.tile([128, 2 * H], BF16, tag="gbrz1", name="gbrz1")
        nc.sync.dma_start(out=gbrz1[:, :], in_=gbrz1d[:, :])
        bhn0 = st.tile([128, H], BF16, tag="bhn0", name="bhn0")
        nc.sync.dma_start(out=bhn0[:, :], in_=bhn0d[:, :])
        bhn1 = st.tile([128, H], BF16, tag="bhn1", name="bhn1")
        nc.sync.dma_start(out=bhn1[:, :], in_=bhn1d[:, :])
        gbn1 = st.tile([128, H], BF16, tag="gbn1", name="gbn1")
        nc.sync.dma_start(out=gbn1[:, :], in_=gbn1d[:, :])
        ident = st.tile([128, BL], BF16, tag="ident", name="ident")
        nc.sync.dma_start(out=ident[:, :], in_=ident_dram[:, :])

        # layer-0 hT ring (4 deep: written slot s, read by L0 at s+1 and
        # by L1 at s+2); layer-1 hT/hF ping-pong; batch-major h state has
        # the two halves stacked at partitions 0 and 32 ([48, 512]).
        hT0 = [st.tile([128, KT, BL], BF16, tag=f"hT0{i}", name=f"hT0{i}")
               for i in range(4)]
        hT1 = [st.tile([128, KT, BL], BF16, tag=f"hT1{i}", name=f"hT1{i}")
               for i in range(2)]
        hF0 = [st.tile([48, HALF], BF16, tag=f"hF0{i}", name=f"hF0{i}")
               for i in range(2)]
        hF1 = [st.tile([48, HALF], BF16, tag=f"hF1{i}", name=f"hF1{i}")
               for i in range(2)]
        # gi0 ring: rows 16.. stay zero so identity folds keep K=128 and
        # the PE never leaves (128,32) tiling mode
        giA = [st.tile([128, G3], BF16, tag=f"gi{i}", name=f"gi{i}")
               for i in range(2)]
        # psum per layer: A = rz (4 chunks x col groups 0-3),
        # B = n recurrent (groups 0,1), C = n input-gate (groups 2,3)
        psA = [pp.tile([128, 512], F32, tag=f"psA{l}", name=f"psA{l}") for l in range(2)]
        psB = [pp.tile([128, 512], F32, tag=f"psB{l}", name=f"psB{l}") for l in range(2)]
        psC = [pp.tile([128, 512], F32, tag=f"psC{l}", name=f"psC{l}") for l in range(2)]
        rga = [st.tile([48, 512], BF16, tag=f"rga{l}", name=f"rga{l}") for l in range(2)]
        zga = [st.tile([48, 512], BF16, tag=f"zga{l}", name=f"zga{l}") for l in range(2)]
        t48 = [st.tile([48, 512], BF16, tag=f"t48{l}", name=f"t48{l}") for l in range(2)]
        n48 = [st.tile([48, 512], BF16, tag=f"n48{l}", name=f"n48{l}") for l in range(2)]

        for l in range(2):
            # one-time init: the wide chain ops read whole banks, incl.
            # the dead rows between the 16-row output slices
            nc.vector.memset(psA[l][:, :], 0.0)
            nc.vector.memset(psB[l][:, :], 0.0)
            nc.vector.memset(psC[l][:, :], 0.0)
        for i in range(4):
            nc.vector.memset(hT0[i][:, :, :], 0.0)
        for i in range(2):
            nc.vector.memset(hT1[i][:, :, :], 0.0)
            nc.vector.memset(hF0[i][:, :], 0.0)
            nc.vector.memset(hF1[i][:, :], 0.0)
            nc.vector.memset(giA[i][:, :], 0.0)
        for s in range(2):
            nc.sync.dma_start(
                out=giA[s][0:BL, :],
                in_=gi_dram[s:s + 1, :, :].rearrange("o b n -> (o b) n"))

        def mm(ps, g, lhsT, rhs, start, stop):
            # parallel accumulation groups live in disjoint 32-partition
            # slices of one bank; the group checker is tile-granular, so
            # skip it (has_written is per-element on HW)
            nc.tensor.matmul(ps[32 * g:32 * g + BL, :], lhsT, rhs,
                             start=start, stop=stop, tile_position=(0, 32 * g),
                             skip_group_check=True)

        def l1_gi_mms(soff):
            # depends only on biases + layer-0 output from two slots ago:
            # leads the slot, hiding the previous slot's chain tails
            h1in = hT0[soff % 4]
            for g in range(4):
                mm(psA[1], g, ident[:, :], gbrz1[:, g * 512:(g + 1) * 512],
                   True, False)
            for g in range(2):
                mm(psB[1], g, ident[:, :], bhn1[:, g * 512:(g + 1) * 512],
                   True, False)
                mm(psC[1], g + 2, ident[:, :], gbn1[:, g * 512:(g + 1) * 512],
                   True, False)
            for k in range(KT):
                last = k == KT - 1
                for g in range(4):
                    mm(psA[1], g, h1in[:, k, :],
                       wih1[:, k, g * 512:(g + 1) * 512], False, False)
                for g in range(2):
                    mm(psC[1], g + 2, h1in[:, k, :],
                       wih1[:, k, 2048 + g * 512:2048 + (g + 1) * 512],
                       False, last)

        def l1_rec_mms(soff):
            hin = hT1[soff % 2]
            for k in range(KT):
                last = k == KT - 1
                for g in range(4):
                    mm(psA[1], g, hin[:, k, :],
                       whh1[:, k, g * 512:(g + 1) * 512], False, last)
                for g in range(2):
                    mm(psB[1], g, hin[:, k, :],
                       whh1[:, k, 2048 + g * 512:2048 + (g + 1) * 512],
                       False, last)

        def l0_mms(soff, hT_r, gi):
            hin = hT0[hT_r]
            for g in range(4):
                mm(psA[0], g, ident[:, :], gi[:, g * 512:(g + 1) * 512],
                   True, False)
            for g in range(2):
                mm(psB[0], g, ident[:, :], bhn0[:, g * 512:(g + 1) * 512],
                   True, False)
                mm(psC[0], g + 2, ident[:, :],
                   gi[:, 2048 + g * 512:2048 + (g + 1) * 512], True, True)
            for k in range(KT):
                last = k == KT - 1
                for g in range(4):
                    mm(psA[0], g, hin[:, k, :],
                       whh0[:, k, g * 512:(g + 1) * 512], False, last)
                for g in range(2):
                    mm(psB[0], g, hin[:, k, :],
                       whh0[:, k, 2048 + g * 512:2048 + (g + 1) * 512],
                       False, last)

        def chain(l, hf_in, hf_out, hout):
            # r gates at psA partitions 0:48, z at 64:112; both sigmoids
            # land at base partition 0 (walrus requires equal SB bases on
            # two-SBUF-input DVE ops; ACT PSUM->SB may cross bases)
            nc.scalar.activation(rga[l][:, :], psA[l][0:48, :], AF.Sigmoid)
            nc.scalar.activation(zga[l][:, :], psA[l][64:112, :], AF.Sigmoid)
            # n-path, halves stacked: rows 0:16 & 32:48 are live lanes
            nc.vector.tensor_mul(t48[l][:, :], rga[l][:, :], psB[l][0:48, :])
            nc.vector.tensor_add(t48[l][:, :], t48[l][:, :], psC[l][64:112, :])
            nc.scalar.activation(n48[l][:, :], t48[l][:, :], AF.Tanh)
            nc.vector.tensor_sub(t48[l][:, :], hf_in[:, :], n48[l][:, :])
            nc.vector.tensor_mul(t48[l][:, :], zga[l][:, :], t48[l][:, :])
            nc.vector.tensor_add(hf_out[:, :], n48[l][:, :], t48[l][:, :])
            # hT refresh on the DMA X-bar ([16,512] -> [128,4,16] per half)
            nc.sync.dma_start_transpose(hout[:, 0:4, :], hf_out[0:BL, :])
            nc.sync.dma_start_transpose(hout[:, 4:8, :], hf_out[32:48, :])

        def slot(t_reg, soff):
            # u = t+soff: layer-1 step u, layer-0 step u+2 (phantom for
            # u+2 >= T: reads gi padding, output unused)
            l1_gi_mms(soff)
            l1_rec_mms(soff)
            l0_mms(soff, (1 + soff) % 4, giA[soff % 2])
            chain(1, hF1[soff % 2], hF1[(soff + 1) % 2], hT1[(soff + 1) % 2])
            chain(0, hF0[soff % 2], hF0[(soff + 1) % 2], hT0[(2 + soff) % 4])
            nc.sync.dma_start(
                out=h2F_dram[ds(t_reg + soff, 1), :, :].rearrange(
                    "o p n -> (o p) n"),
                in_=hF1[(soff + 1) % 2][:, :])
            nc.sync.dma_start(
                out=giA[soff % 2][0:BL, :],
                in_=gi_dram[ds(t_reg + soff + 4, 1), :, :].rearrange(
                    "o b n -> (o b) n"))

        # prologue: layer-0 steps 0 and 1 (no layer-1 work yet)
        for s in range(2):
            l0_mms(s, (s + 3) % 4, giA[s % 2])
            chain(0, hF0[s % 2], hF0[(s + 1) % 2], hT0[s % 4])
            nc.sync.dma_start(
                out=giA[s % 2][0:BL, :],
                in_=gi_dram[s + 2:s + 3, :, :].rearrange("o b n -> (o b) n"))

        # loop register u = t+soff is the LAYER-1 step; layer-0 runs
        # step u+2 (all register offsets stay non-negative: negative
        # addends in ds() break walrus DMA AP lowering)
        with tc.For_i(0, T, 8) as t:
            for soff in range(8):
                slot(t, soff)


# ---------------- host-side wrapper ----------------
def _prep_inputs(x, w_ih0, w_hh0, b_ih0, b_hh0, w_ih1, w_hh1, b_ih1, b_hh1,
                 w_lin, b_lin, T):
    NTOK = T * BL
    bf = ml_dtypes.bfloat16

    def prep_layer(w_ih, w_hh, b_ih, b_hh):
        # torch gate order [r; z; n] is already the kernel's chunk order
        w_ihT = np.ascontiguousarray(np.asarray(w_ih).T).astype(bf)
        w_hhT = np.ascontiguousarray(np.asarray(w_hh).T).astype(bf).reshape(
            KT, 128, G3)
        b_ihP = np.asarray(b_ih)
        b_hhP = np.asarray(b_hh)
        gb = b_ihP.copy()
        gb[:2 * H] += b_hhP[:2 * H]      # r/z: both biases pre-summed
        bhn = b_hhP[2 * H:]              # n: b_hh inside the r*() term
        gbn = b_ihP[2 * H:]              # n: b_ih outside it
        return w_ihT, w_hhT, gb, bhn, gbn

    w_ih0T, w_hh0T, gb0, bhn0, _ = prep_layer(w_ih0, w_hh0, b_ih0, b_hh0)
    w_ih1T, w_hh1T, gb1, bhn1, gbn1 = prep_layer(w_ih1, w_hh1, b_ih1, b_hh1)
    w_ih1T = w_ih1T.reshape(KT, 128, G3)
    w_linT = np.ascontiguousarray(np.asarray(w_lin).T).astype(bf).reshape(
        KT, 128, OUT)
    blinB = np.broadcast_to(np.asarray(b_lin).astype(np.float32),
                            (128, OUT)).copy()
    identB = np.eye(128, BL, dtype=bf)

    def bc(v, dt=bf):
        return np.broadcast_to(np.asarray(v).astype(dt), (128, len(v))).copy()

    common = dict(
        w_ih0T=w_ih0T, w_hh0T=w_hh0T, w_ih1T=w_ih1T, w_hh1T=w_hh1T,
        w_linT=w_linT,
        gbias0=bc(gb0, np.float32),
        gbrz1=bc(gb1[:2 * H]),
        bhn0=bc(bhn0), bhn1=bc(bhn1), gbn1=bc(gbn1),
        blin=blinB, identB=identB)

    in_maps = []
    x = np.asarray(x)
    for c in range(N_CORES):
        xs = x[:T, c * BL:(c + 1) * BL, :]
        xT = np.zeros((IN, NTOK + 128), dtype=bf)
        xT[:, :NTOK] = xs.reshape(NTOK, IN).T.astype(bf)
        m = dict(common)
        m["xT"] = xT
        in_maps.append(m)
    return in_maps


_NC_CACHE = {}


def get_program(T):
    if T not in _NC_CACHE:
        _NC_CACHE[T] = _build_program(T)
    return _NC_CACHE[T]


def run(x, w_ih0, w_hh0, b_ih0, b_hh0, w_ih1, w_hh1, b_ih1, b_hh1,
        w_lin, b_lin, T=512, trace=False):
    in_maps = _prep_inputs(x, w_ih0, w_hh0, b_ih0, b_hh0, w_ih1, w_hh1,
                           b_ih1, b_hh1, w_lin, b_lin, T)
    nc = get_program(T)
    res = run_bass_kernel_spmd(nc, in_maps, core_ids=list(range(N_CORES)),
                               trace=trace)
    out = np.empty((T, B, OUT), dtype=np.float32)
    for c in range(N_CORES):
        out[:, c * BL:(c + 1) * BL, :] = res.results[c]["y"].reshape(
            T, BL, OUT)
    return out, res


def kernel(**inputs):
    out, _ = run(**inputs)
    return out
